# revision 67
# baseline (speedup 1.0000x reference)
"""ChebNet (K=4, two layers, log_softmax) on 8 Trainium2 NeuronCores.

Self-contained: takes FULL inputs, shards by destination node internally,
runs a single SPMD Bass kernel on cores 0-7, returns the FULL [N, 32]
output.

Math (Horner re-association so propagations happen at output width):
    y = sum_k T_k(L) x @ W[k] + b,  T_k Chebyshev,  L = -D^-1/2 A D^-1/2
      = U0 + L(U1 + L(U2 + L U3)),  U_j = x @ V_j
    V0 = W0 - W2, V1 = W1 - 3 W3, V2 = 2 W2, V3 = 4 W3
Scaled-space recurrence (tables hold S_hat = dis * S, dis = rsqrt degree):
    S_hat3 = dis*U3;  S_hat_j = dis*U_j - dis^2 * (A S_hat_{j+1})
    final: y = U_0 + b - dis * (A S_hat_1)

Per hop, A S_hat is an edge-gather + segment-sum:
    SWDGE dma_gather (8-chunk calls, the HW ring cap, round-robined over
    4 Q7 queues) -> bf16 one-hot (seg == iota) built on VectorE ->
    TensorE bf16 matmul accumulation into PSUM per 128-dest tile.

The gather table for each hop is PIECE-MAJOR across TWO tensors: the
producing hop AllGathers piece 0 (groups 0-6) mid-hop and piece 1 at its
end; gather buckets are piece-aligned, so the consuming hop's bucket-0
calls depend only on piece 0 and are emitted first (bridge), keeping the
Pool engine busy across the hop transition while piece 1's AllGather
finishes.  U_1/U_2 tables live in SBUF as bf16 (U_0, needed only by the
final hop of each layer, round-trips through HBM to free SBUF for a
deeper gather-tile ring); the layer-2 prologue (h^T @ V2) is fused into
layer 1's final writeout.
Gathers are nosync-chained in issue order so the scheduler cannot invert
SWDGE ring-FIFO order or the gather-pool ring release order.
"""

import sys

if "/opt/trn_rl_repo" not in sys.path:
    sys.path.insert(0, "/opt/trn_rl_repo")

import math
from contextlib import ExitStack
from dataclasses import dataclass, field

import numpy as np

P = 128
GCH = 8        # chunks per dma_gather call (65 descs/engine = HW SWDGE ring cap; 9+ hangs)
OB = 16        # one-hot pairs per DVE batch
GT = 8         # dest tiles per psum group
NQ = 4         # SWDGE queues
N_NODES = 100000
N_CORES = 8
CIN, HID, OUT = 128, 64, 32


@dataclass
class LayerSched:
    q: int                 # nodes per 256B gather row
    w: int                 # data width (channels)
    nb: int                # buckets
    buckrows: int          # unused (buckets are piece-aligned; see buckoff)
    buckoff: list = field(default_factory=list)  # per-bucket q-row offsets
    nch: int = 0           # total chunks
    npair: int = 0         # total one-hot pairs
    # per group g: list of calls (bucket, chunk0, glen)
    calls: list = field(default_factory=list)
    # per group g: list of pairs (chunk, tile_local, segcol, par, start, stop)
    pairs: list = field(default_factory=list)


@dataclass
class Cfg:
    n: int
    m: int
    b: int = 0
    t: int = 0
    bp: int = 0
    ng: int = 0
    nch: int = 0
    L: list = field(default_factory=list)  # [LayerSched x2]

    def finish(self):
        assert self.n % self.m == 0
        self.b = self.n // self.m
        self.t = (self.b + P - 1) // P
        self.bp = self.t * P
        self.ng = (self.t + GT - 1) // GT


def pieces(cfg: Cfg):
    """Chunked-AllGather piece boundaries (in dest-row units per core).

    The global gather table is piece-major, ONE TENSOR PER PIECE: piece p's
    AllGather launches as soon as the producing hop finishes groups
    [PG[p], PG[p+1]); gather buckets are piece-aligned, so the consuming
    hop's bucket-p gathers depend only on piece p — they bridge the
    inter-hop bubble."""
    PG = [0, 7, cfg.ng]
    pr = [
        (PG[p] * GT * P, min(PG[p + 1] * GT * P, cfg.bp))
        for p in range(len(PG) - 1)
    ]
    return PG, pr


def _layer_sched(cfg: Cfg, row, col, q, w):
    """Build the edge-stream schedule for one layer (shared across cores)."""
    m, b, bp, t, ng = cfg.m, cfg.b, cfg.bp, cfg.t, cfg.ng

    _, pr = pieces(cfg)
    pstarts = np.array([r0 for r0, _ in pr], dtype=np.int64)
    pends = np.array([r1 for _, r1 in pr], dtype=np.int64)
    pbase = np.concatenate([[0], np.cumsum((pends - pstarts) * m)])

    # bucket 0 = layout piece 0; bucket 1 = layout pieces 1+2 (whose
    # AllGathers are split so the last one is small)
    nb = 2
    buckoff = [0, int(pbase[1]) // q, int(pbase[-1]) // q]
    for p in range(nb):
        assert buckoff[p + 1] - buckoff[p] <= 32767

    ls = LayerSched(q=q, w=w, nb=nb, buckrows=0)
    ls.buckoff = buckoff
    ncell = ng * nb * q

    per_core = []
    counts = np.zeros((m, ncell), dtype=np.int64)
    for c in range(m):
        sel = (row >= c * b) & (row < (c + 1) * b)
        d = (row[sel] - c * b).astype(np.int64)
        s = col[sel].astype(np.int64)
        # piece-major table layout: (piece, core, local row within piece)
        sc = s // b
        rl = s % b
        pidx = np.searchsorted(pends, rl, side="right")
        trow = pbase[pidx] + sc * (pends[pidx] - pstarts[pidx]) + (
            rl - pstarts[pidx]
        )
        prq = trow // q
        par = trow % q
        buck = np.minimum(pidx, 1)
        lidx = prq - np.array(buckoff, dtype=np.int64)[buck]
        tile = d >> 7
        g = tile // GT
        cid = (buck * ng + g) * q + par
        order = np.lexsort((tile, cid))
        per_core.append((d[order], lidx[order], cid[order], tile[order]))
        counts[c] = np.bincount(cid, minlength=ncell)

    kcell = np.array(
        [math.ceil(int(counts[:, i].max()) / P) for i in range(ncell)],
        dtype=np.int64,
    )

    # global chunk layout: cells in cid order; chunk -> (g, buck, par)
    cell_chunk0 = np.concatenate([[0], np.cumsum(kcell)])
    nch = int(cell_chunk0[-1])
    ls.nch = nch

    # per-core slot streams
    S = nch * P
    idx = np.zeros((m, S), dtype=np.int32)
    dloc = np.full((m, S), -1, dtype=np.int64)   # dest local id, -1 pad
    dtile = np.full((m, S), -1, dtype=np.int64)
    for c in range(m):
        d, lidx, cid, tile = per_core[c]
        pos_in_cell = np.arange(d.size) - np.concatenate(
            [[0], np.cumsum(counts[c])]
        )[cid]
        slot = cell_chunk0[cid] * P + pos_in_cell
        idx[c, slot] = lidx
        dloc[c, slot] = d
        dtile[c, slot] = tile

    # calls: bucket-pure GCH windows over each bucket's GLOBALLY contiguous
    # chunk range (cells are bucket-major, so group boundaries don't
    # fragment calls); annotated with the first chunk's group for emission
    # ordering. pairs: per chunk, union tile range over cores, TILE-MAJOR.
    def chunk_group(k):
        cid = int(np.searchsorted(cell_chunk0, k, side="right")) - 1
        return (cid // q) % ng

    ls.flatcalls = []
    for buck in range(nb):
        c0 = int(cell_chunk0[(buck * ng + 0) * q])
        cl_ = (buck * ng + (ng - 1)) * q + q - 1
        c1 = int(cell_chunk0[cl_] + kcell[cl_])
        for w0 in range(c0, c1, GCH):
            glen = min(GCH, c1 - w0)
            ls.flatcalls.append((buck, w0, glen, chunk_group(w0)))

    for g in range(ng):
        gpairs = []
        tcnt = min(GT, t - g * GT)
        for par in range(q):
            for buck in range(nb):
                cid = (buck * ng + g) * q + par
                k0 = int(cell_chunk0[cid])
                k1 = int(cell_chunk0[cid] + kcell[cid])
                for k in range(k0, k1):
                    tl = dtile[:, k * P : (k + 1) * P]
                    real = tl >= 0
                    if not real.any():
                        continue
                    lo = int(tl[real].min())
                    hi = int(tl[real].max())
                    for tt in range(lo, hi + 1):
                        gpairs.append([k, tt - g * GT, par, 0, False, False])
        gpairs.sort(key=lambda e: (e[1], e[0]))
        first = {}
        last = {}
        for i, e in enumerate(gpairs):
            key = e[1]
            if key not in first:
                first[key] = i
            last[key] = i
        for key, i in first.items():
            gpairs[i][4] = True
        for key, i in last.items():
            gpairs[i][5] = True
        ls.pairs.append(gpairs)

    ls.npair = sum(len(p) for p in ls.pairs)
    ls._idx, ls._dloc, ls._dtile = idx, dloc, dtile
    ls._cell_chunk0, ls._kcell = cell_chunk0, kcell

    # per-core seg matrix [P, npair] and idx16 stream
    seg_all, idx_all = [], []
    for c in range(m):
        seg = np.full((P, ls.npair), -1.0, dtype=np.float32)
        colp = 0
        for g in range(ng):
            for k, ttl, par, cc, st, sp in ls.pairs[g]:
                tt = ttl + g * GT
                tl = dtile[c, k * P : (k + 1) * P]
                dl = dloc[c, k * P : (k + 1) * P]
                mask = tl == tt
                seg[mask, colp] = (dl[mask] & 127).astype(np.float32)
                colp += 1
        seg_all.append(np.ascontiguousarray(seg))

        # idx16: slot i -> partition i%16 col i//16, replicated x8
        i16 = idx[c].astype(np.int16).reshape(S // 16, 16).T
        idx_all.append(np.ascontiguousarray(np.tile(i16, (8, 1))))
    return ls, idx_all, seg_all


def preprocess(edge_index: np.ndarray, cfg: Cfg):
    row = np.asarray(edge_index[0], dtype=np.int64)
    col = np.asarray(edge_index[1], dtype=np.int64)
    deg = np.bincount(row, minlength=cfg.n).astype(np.float32)

    l1, idx1, seg1 = _layer_sched(cfg, row, col, q=2, w=HID)
    l2, idx2, seg2 = _layer_sched(cfg, row, col, q=4, w=OUT)
    cfg.L = [l1, l2]
    cfg.nch = l1.nch

    degt_all = []
    for c in range(cfg.m):
        degb = np.zeros(cfg.bp, dtype=np.float32)
        degb[: cfg.b] = deg[c * cfg.b : (c + 1) * cfg.b]
        degt_all.append(np.ascontiguousarray(degb.reshape(cfg.t, P).T))
    return (idx1, seg1, idx2, seg2), degt_all


def build_program(cfg: Cfg):
    import ml_dtypes
    import concourse.bass as bass
    import concourse.tile as tile
    from concourse import bacc, mybir

    f32 = mybir.dt.float32
    bf16 = mybir.dt.bfloat16
    i16 = mybir.dt.int16
    m, b, T, bp, ng = cfg.m, cfg.b, cfg.t, cfg.bp, cfg.ng
    NTAB = m * bp
    l1, l2 = cfg.L

    nc = bacc.Bacc(
        "TRN2", target_bir_lowering=False, debug=False, num_devices=m,
        num_swdge_queues=NQ,
    )

    # ---- I/O ----
    x_t = nc.dram_tensor("xT_blk", [CIN, bp], bf16, kind="ExternalInput")
    w1_t = nc.dram_tensor("W1", [4, CIN, HID], f32, kind="ExternalInput")
    b1_t = nc.dram_tensor("b1", [HID], f32, kind="ExternalInput")
    w2_t = nc.dram_tensor("W2", [4, HID, OUT], f32, kind="ExternalInput")
    b2_t = nc.dram_tensor("b2", [OUT], f32, kind="ExternalInput")
    degt_t = nc.dram_tensor("deg_t", [P, T], f32, kind="ExternalInput")
    idx1_t = nc.dram_tensor("idx1", [P, l1.nch * 8], i16, kind="ExternalInput")
    seg1_t = nc.dram_tensor("seg1", [P, l1.npair], bf16, kind="ExternalInput")
    idx2_t = nc.dram_tensor("idx2", [P, l2.nch * 8], i16, kind="ExternalInput")
    seg2_t = nc.dram_tensor("seg2", [P, l2.npair], bf16, kind="ExternalInput")
    y_t = nc.dram_tensor("y_blk", [bp, OUT], f32, kind="ExternalOutput")

    # ---- internal DRAM: per-piece blk tensors + single tab per hop ----
    # pieces cut the dest-row space at group granularity so each piece's
    # AllGather can launch as soon as its last group's writeout lands,
    # overlapping collective traffic with the producing hop
    PG, _pr = pieces(cfg)
    NPC = len(PG) - 1
    # piece p's table row base (dest-row x m units; piece-major layout)
    pbase = [0]
    for p in range(NPC):
        pbase.append(pbase[-1] + (_pr[p][1] - _pr[p][0]) * m)

    def prows(p):  # dest-row range of piece p
        return _pr[p]

    u0_1 = nc.dram_tensor("U0_1", [bp, HID], bf16)
    u0_2 = nc.dram_tensor("U0_2", [bp, OUT], bf16)
    # write pieces: piece 0 = groups 0-6, piece 1 = groups 7-12 (layout
    # pieces 1+2 merged -> ONE tail AllGather per hop: each collective pays
    # an inter-core skew wait, so fewer, bigger AGs cost less)
    WPR = [prows(0), prows(1)]
    blks, tabs = {}, {}
    for l, ls in ((1, l1), (2, l2)):
        for j in (3, 2, 1):
            for p in range(2):
                r0, r1 = WPR[p]
                blks[(l, j, p)] = nc.dram_tensor(
                    f"blk_{l}_{j}_{p}", [(r1 - r0) // ls.q, 128], bf16
                )
            # one tab tensor PER BUCKET: bucket 0 <- piece 0's AllGather;
            # bucket 1 <- pieces 1+2, two AllGathers into adjacent ranges
            tabs[(l, j, 0)] = nc.dram_tensor(
                f"tab_{l}_{j}_0",
                [(prows(0)[1] - prows(0)[0]) * m // ls.q, 128], bf16,
                addr_space="Shared",
            )
            tabs[(l, j, 1)] = nc.dram_tensor(
                f"tab_{l}_{j}_1",
                [(prows(1)[1] - prows(1)[0]) * m // ls.q, 128], bf16,
                addr_space="Shared",
            )

    iota_np = np.broadcast_to(
        np.tile(np.arange(P, dtype=np.float32), OB), (P, OB * P)
    ).astype(ml_dtypes.bfloat16)
    iota_d = nc.inline_tensor(iota_np, name="iota_rep")
    ident_d = nc.inline_tensor(np.eye(P, dtype=np.float32), name="ident")

    with ExitStack() as ctx:
        tc = ctx.enter_context(tile.TileContext(nc, num_cores=m))
        const = ctx.enter_context(tc.tile_pool(name="const", bufs=1))
        xp = ctx.enter_context(tc.tile_pool(name="xp", bufs=2))
        wp = ctx.enter_context(tc.tile_pool(name="wp", bufs=3))
        gp = ctx.enter_context(tc.tile_pool(name="gp", bufs=33))
        up = ctx.enter_context(tc.tile_pool(name="up", bufs=2))
        op = ctx.enter_context(tc.tile_pool(name="op", bufs=3))
        ep = ctx.enter_context(tc.tile_pool(name="ep", bufs=2))
        pst = ctx.enter_context(tc.tile_pool(name="pst", bufs=2, space="PSUM"))
        psu = ctx.enter_context(tc.tile_pool(name="psu", bufs=3, space="PSUM"))
        psa = ctx.enter_context(tc.tile_pool(name="psa", bufs=3, space="PSUM"))

        # ---- constants ----
        iota_s = const.tile([P, OB * P], bf16)
        nc.sync.dma_start(iota_s[:], iota_d[:, :])
        ident_s = const.tile([P, P], f32)
        nc.sync.dma_start(ident_s[:], ident_d[:, :])

        idx_s = const.tile([P, max(l1.nch, l2.nch) * 8], i16)
        nc.sync.dma_start(idx_s[:, : l1.nch * 8], idx1_t[:, :])
        seg1_s = const.tile([P, l1.npair], bf16)
        nc.sync.dma_start(seg1_s[:], seg1_t[:, :])
        seg2_s = const.tile([P, l2.npair], bf16)
        nc.sync.dma_start(seg2_s[:], seg2_t[:, :])

        # U_j (j=1,2) tables resident in SBUF (bf16): [P, 2, T, w]; the
        # U_0 slices live in HBM (u0_l), read back only by the final hop
        U1s = const.tile([P, 2, T, HID], bf16)
        U2s = const.tile([P, 2, T, OUT], bf16)

        # early slice of the layer-2 idx stream (groups 0-1 + g2 bucket 0),
        # loaded at program start into its own tile: the L2 bridge gathers
        # don't wait for the post-layer-1 idx_s reload (WAR on idx_s)
        l2_thresh = int(l2._cell_chunk0[(2 * l2.nb + 0) * l2.q + l2.q - 1]
                        + l2._kcell[(2 * l2.nb + 0) * l2.q + l2.q - 1])
        idx2b = const.tile([P, l2_thresh * 8], i16)
        nc.sync.dma_start(idx2b[:], idx2_t[:, : l2_thresh * 8])

        # V1cat [CIN, 4, HID], V2cat [HID, 4, OUT]
        def vcat(w_t, cl, w):
            ws = const.tile([cl, 4, w], f32)
            nc.sync.dma_start(ws[:], w_t[:, :, :].rearrange("k p c -> p k c"))
            v = const.tile([cl, 4, w], f32)
            nc.vector.tensor_sub(v[:, 0, :], ws[:, 0, :], ws[:, 2, :])
            nc.vector.tensor_scalar(
                out=v[:, 1, :], in0=ws[:, 3, :], scalar1=-3.0, scalar2=None,
                op0=mybir.AluOpType.mult,
            )
            nc.vector.tensor_add(v[:, 1, :], v[:, 1, :], ws[:, 1, :])
            nc.vector.tensor_scalar(
                out=v[:, 2, :], in0=ws[:, 2, :], scalar1=2.0, scalar2=None,
                op0=mybir.AluOpType.mult,
            )
            nc.vector.tensor_scalar(
                out=v[:, 3, :], in0=ws[:, 3, :], scalar1=4.0, scalar2=None,
                op0=mybir.AluOpType.mult,
            )
            return v

        v1f = vcat(w1_t, CIN, HID)
        v2f = vcat(w2_t, HID, OUT)
        v1 = const.tile([CIN, 4, HID], bf16)
        nc.vector.tensor_copy(v1[:], v1f[:])
        v2 = const.tile([HID, 4, OUT], bf16)
        nc.vector.tensor_copy(v2[:], v2f[:])
        ident_b = const.tile([P, P], bf16)
        nc.vector.tensor_copy(ident_b[:], ident_s[:])

        b1s = const.tile([P, HID], f32)
        nc.sync.dma_start(b1s[:1, :], b1_t[:].rearrange("(o c) -> o c", o=1))
        nc.gpsimd.partition_broadcast(b1s[:, :], b1s[:1, :])
        b2s = const.tile([P, OUT], f32)
        nc.sync.dma_start(b2s[:1, :], b2_t[:].rearrange("(o c) -> o c", o=1))
        nc.gpsimd.partition_broadcast(b2s[:, :], b2s[:1, :])

        # ---- dis, -dis, -dis^2 in [P, T]: (p, t) = dest 128t+p ----
        degs = const.tile([P, T], f32)
        nc.sync.dma_start(degs[:], degt_t[:, :])
        dis = const.tile([P, T], f32)
        ndis = const.tile([P, T], f32)
        ndis2 = const.tile([P, T], f32)
        tmp = const.tile([P, T], f32)
        nc.vector.tensor_scalar(
            out=tmp[:], in0=degs[:], scalar1=1.0, scalar2=None,
            op0=mybir.AluOpType.max,
        )
        nc.scalar.activation(tmp[:], tmp[:], mybir.ActivationFunctionType.Sqrt)
        nc.vector.reciprocal(dis[:], tmp[:])
        nc.vector.tensor_scalar(
            out=tmp[:], in0=degs[:], scalar1=0.0, scalar2=None,
            op0=mybir.AluOpType.is_gt,
        )
        nc.vector.tensor_mul(dis[:], dis[:], tmp[:])
        nc.vector.tensor_scalar(
            out=ndis[:], in0=dis[:], scalar1=-1.0, scalar2=None,
            op0=mybir.AluOpType.mult,
        )
        nc.vector.tensor_mul(ndis2[:], dis[:], ndis[:])

        # blk row views: [rows, w] over [rows/q, 128]
        def rows_view(blk, w):
            return blk[:, :].rearrange("r (t c) -> (r t) c", c=w)

        def piece_of_group(g):
            return 0 if g < PG[1] else 1

        def allgather_piece(l, j, p, q):
            nc.gpsimd.collective_compute(
                "AllGather",
                mybir.AluOpType.bypass,
                replica_groups=[list(range(m))],
                ins=[blks[(l, j, p)].ap().opt()],
                outs=[tabs[(l, j, p)].ap().opt()],
            )

        # per-(l, j) piece row views
        BV = {
            (l, j): [
                rows_view(blks[(l, j, p)], w) for p in range(2)
            ]
            for (l, w) in ((1, HID), (2, OUT))
            for j in (3, 2, 1)
        }

        def piece_write(l, j, g, tcnt, bt):
            """DMA one group's [P, tcnt, w] rows into its blk piece; kick the
            piece's AllGather when this was the piece's last group."""
            pidx = piece_of_group(g)
            pr0, _ = WPR[pidx]
            loc = slice(g * GT * P - pr0, g * GT * P - pr0 + tcnt * P)
            nc.sync.dma_start(
                BV[(l, j)][pidx][loc, :].rearrange("(a p) c -> p a c", p=P),
                bt[:, :tcnt, :],
            )
            if g == (PG[1] - 1 if pidx == 0 else ng - 1):
                allgather_piece(l, j, pidx, l1.q if l == 1 else l2.q)

        # ---- prologue: U_j = x @ V_j; U1 j=0 (+bias), j=1,2 (dis-scaled)
        #      -> SBUF; j=3 dis-scaled -> blk_1_3 pieces (+AllGather) ----
        def prologue():
            for g in range(ng):
                tcnt = min(GT, T - g * GT)
                xT = xp.tile([CIN, GT * P], bf16, tag="xTd")
                nc.sync.dma_start(
                    xT[:, : tcnt * P],
                    x_t[:, g * GT * P : (g * GT + tcnt) * P],
                )
                btg = ep.tile([P, GT, HID], bf16, tag="bt")
                u0g = up.tile([P, GT, HID], bf16, tag="u0g")
                for a in range(tcnt):
                    k = g * GT + a
                    upsum = psu.tile([P, 4, HID], f32, space="PSUM", tag="upsum")
                    nc.tensor.matmul(
                        out=upsum[:].rearrange("p a c -> p (a c)"),
                        lhsT=xT[:, a * P : (a + 1) * P],
                        rhs=v1[:].rearrange("p a c -> p (a c)"),
                        start=True, stop=True,
                    )
                    nc.vector.tensor_add(
                        u0g[:, a, :], upsum[:, 0, :], b1s[:, :HID]
                    )
                    nc.vector.tensor_scalar(
                        out=U1s[:, 0:2, k, :], in0=upsum[:, 1:3, :],
                        scalar1=dis[:, k : k + 1], scalar2=None,
                        op0=mybir.AluOpType.mult,
                    )
                    nc.vector.tensor_scalar(
                        out=btg[:, a, :], in0=upsum[:, 3, :],
                        scalar1=dis[:, k : k + 1], scalar2=None,
                        op0=mybir.AluOpType.mult,
                    )
                nc.sync.dma_start(
                    u0_1[g * GT * P : (g * GT + tcnt) * P, :].rearrange(
                        "(a p) c -> p a c", p=P
                    ),
                    u0g[:, :tcnt, :],
                )
                piece_write(1, 3, g, tcnt, btg)

        # ---- one hop: consume per-piece tabs (l, jin); produce blk pieces
        #      (l, jout) or the final output ----
        def hop(ls, idx_s, seg_s, jin, Us, jidx, final, l, idx_early=None,
                early_thresh=0):
            w = ls.w
            segbase = 0
            qrr = [0]

            # gather emission order: BRIDGE first — bucket-0 calls (which
            # depend only on piece 0 of the input table, AllGathered mid-way
            # through the PREVIOUS hop) keep the Pool engine busy across the
            # inter-hop transition — then (group, bucket)-major remainder.
            bridge_budget = 24
            order, taken = [], set()
            for ci, (buck, w0, glen, gf) in enumerate(ls.flatcalls):
                if buck == 0 and len(order) < bridge_budget:
                    order.append(ci)
                    taken.add(ci)
            rest = [ci for ci in range(len(ls.flatcalls)) if ci not in taken]
            rest.sort(key=lambda ci: (ls.flatcalls[ci][3], ls.flatcalls[ci][0]))
            order += rest

            gath = {}
            for ci in order:
                buck, w0, glen, gf = ls.flatcalls[ci]
                gt = gp.tile([P, GCH, 128], bf16, tag="gath")
                isrc = (idx_early if idx_early is not None
                        and w0 + glen <= early_thresh else idx_s)
                gi = nc.gpsimd.dma_gather(
                    out_ap=gt[:, :glen, :],
                    in_ap=tabs[(l, jin, buck)][:, :],
                    idxs_ap=isrc[:, w0 * 8 : (w0 + glen) * 8],
                    num_idxs=glen * P,
                    num_idxs_reg=glen * P,
                    elem_size=128,
                    queue_num=qrr[0] % NQ,
                )
                # chain gathers in issue order: keeps engine order consistent
                # with the gp-ring release order (and SWDGE FIFO order)
                if last_gather[0] is not None:
                    gi.ins.add_dependency(
                        last_gather[0],
                        mybir.DependencyInfo(sync=False, no_sync=True),
                    )
                last_gather[0] = gi.ins.name
                qrr[0] += 1
                for j in range(glen):
                    gath[w0 + j] = (gt, j)

            for g in range(ng):
                tcnt = min(GT, T - g * GT)
                psum = psa.tile([P, GT, w], f32, space="PSUM", tag="apsum")
                started = {e[1] for e in ls.pairs[g] if e[4]}
                for ttl in range(tcnt):
                    if ttl not in started:
                        nc.vector.memset(psum[:, ttl, :], 0.0)
                oneh = None
                npair_g = len(ls.pairs[g])
                for i, (k, ttl, par, cc, st, sp) in enumerate(ls.pairs[g]):
                    opos = i % OB
                    if opos == 0:
                        olen = min(OB, npair_g - i)
                        oneh = op.tile([P, OB, P], bf16, tag="oneh")
                        nc.vector.tensor_tensor(
                            out=oneh[:, :olen, :],
                            in0=iota_s[:].rearrange("p (a q) -> p a q", q=P)[
                                :, :olen, :
                            ],
                            in1=seg_s[:, segbase + i : segbase + i + olen]
                            .to_broadcast([P, olen, P]),
                            op=mybir.AluOpType.is_equal,
                        )
                    gt, slot = gath[k]
                    nc.tensor.matmul(
                        out=psum[:, ttl, :],
                        lhsT=oneh[:, opos, :],
                        rhs=gt[:, slot, par * w : (par + 1) * w],
                        start=st, stop=sp,
                    )
                segbase += npair_g

                # ---- writeout ----
                sl = slice(g * GT, g * GT + tcnt)
                wt = wp.tile([P, GT, w], f32, tag="wt")
                nc.vector.tensor_tensor(
                    out=wt[:, :tcnt, :],
                    in0=psum[:, :tcnt, :],
                    in1=(ndis if final else ndis2)[:, sl].to_broadcast(
                        [P, tcnt, w]
                    ),
                    op=mybir.AluOpType.mult,
                )
                if final:
                    u0t = up.tile([P, GT, w], bf16, tag="u0t")
                    nc.sync.dma_start(
                        u0t[:, :tcnt, :],
                        (u0_1 if l == 1 else u0_2)[
                            g * GT * P : (g * GT + tcnt) * P, :
                        ].rearrange("(a p) c -> p a c", p=P),
                    )
                    nc.vector.tensor_add(
                        wt[:, :tcnt, :], wt[:, :tcnt, :], u0t[:, :tcnt, :]
                    )
                else:
                    nc.vector.tensor_add(
                        wt[:, :tcnt, :], wt[:, :tcnt, :],
                        Us[:, jidx - 1, sl, :],
                    )
                if not final:
                    bt = ep.tile([P, GT, w], bf16, tag="bt")
                    nc.vector.tensor_copy(bt[:, :tcnt, :], wt[:, :tcnt, :])
                    piece_write(l, jidx, g, tcnt, bt)
                elif l == 1:
                    nc.vector.tensor_scalar(
                        out=wt[:, :tcnt, :], in0=wt[:, :tcnt, :],
                        scalar1=0.0, scalar2=None, op0=mybir.AluOpType.max,
                    )
                    bt = ep.tile([P, GT, w], bf16, tag="btr")
                    nc.vector.tensor_copy(bt[:, :tcnt, :], wt[:, :tcnt, :])
                    # fused layer-2 prologue: U2_j (SBUF) and blk_2_3 pieces
                    # straight from the relu'd SBUF tiles
                    bt2g = wp.tile([P, GT, OUT], bf16, tag="bt2")
                    u0g2 = up.tile([P, GT, OUT], bf16, tag="u0g2")
                    for a in range(tcnt):
                        k = g * GT + a
                        tp = pst.tile([HID, P], bf16, space="PSUM", tag="tp")
                        nc.tensor.transpose(
                            out=tp[:, :], in_=bt[:, a, :], identity=ident_b[:]
                        )
                        hT = wp.tile([HID, P], bf16, tag="xT")
                        nc.vector.tensor_copy(hT[:], tp[:, :])
                        upsum = psu.tile(
                            [P, 4, OUT], f32, space="PSUM", tag="upsum"
                        )
                        nc.tensor.matmul(
                            out=upsum[:].rearrange("p a c -> p (a c)"),
                            lhsT=hT[:, :],
                            rhs=v2[:].rearrange("p a c -> p (a c)"),
                            start=True, stop=True,
                        )
                        nc.vector.tensor_add(
                            u0g2[:, a, :], upsum[:, 0, :], b2s[:, :OUT]
                        )
                        nc.vector.tensor_scalar(
                            out=U2s[:, 0:2, k, :], in0=upsum[:, 1:3, :],
                            scalar1=dis[:, k : k + 1], scalar2=None,
                            op0=mybir.AluOpType.mult,
                        )
                        nc.vector.tensor_scalar(
                            out=bt2g[:, a, :], in0=upsum[:, 3, :],
                            scalar1=dis[:, k : k + 1], scalar2=None,
                            op0=mybir.AluOpType.mult,
                        )
                    nc.sync.dma_start(
                        u0_2[g * GT * P : (g * GT + tcnt) * P, :].rearrange(
                            "(a p) c -> p a c", p=P
                        ),
                        u0g2[:, :tcnt, :],
                    )
                    piece_write(2, 3, g, tcnt, bt2g)
                else:
                    nc.vector.tensor_copy(lsm[:, sl, :], wt[:, :tcnt, :])

        lsm = const.tile([P, T, OUT], f32)
        last_gather = [None]

        # ================= layer 1 =================
        prologue()
        for j in (2, 1, 0):
            hop(l1, idx_s, seg1_s, j + 1, U1s, j,
                final=(j == 0), l=1)

        # ================= layer 2 =================
        nc.sync.dma_start(idx_s[:, : l2.nch * 8], idx2_t[:, :])
        for j in (2, 1, 0):
            hop(l2, idx_s, seg2_s, j + 1, U2s, j,
                final=(j == 0), l=2, idx_early=idx2b,
                early_thresh=l2_thresh)

        # ---- batched log_softmax epilogue over lsm [P, T, OUT] ----
        red = const.tile([P, T], f32)
        nc.vector.tensor_reduce(
            out=red[:], in_=lsm[:, :, :], axis=mybir.AxisListType.X,
            op=mybir.AluOpType.max,
        )
        nc.vector.tensor_tensor(
            out=lsm[:, :, :], in0=lsm[:, :, :],
            in1=red[:].to_broadcast([P, T, OUT]),
            op=mybir.AluOpType.subtract,
        )
        ex = const.tile([P, T, OUT], bf16)
        nc.scalar.activation(ex[:], lsm[:, :, :], mybir.ActivationFunctionType.Exp)
        nc.vector.tensor_reduce(
            out=red[:], in_=ex[:, :, :], axis=mybir.AxisListType.X,
            op=mybir.AluOpType.add,
        )
        nc.scalar.activation(red[:], red[:], mybir.ActivationFunctionType.Ln)
        nc.vector.tensor_tensor(
            out=lsm[:, :, :], in0=lsm[:, :, :],
            in1=red[:].to_broadcast([P, T, OUT]),
            op=mybir.AluOpType.subtract,
        )
        nc.sync.dma_start(
            y_t[:, :].rearrange("(a p) c -> p a c", p=P), lsm[:, :, :]
        )


    nc.compile()
    return nc


def make_in_maps(cfg: Cfg, inputs: dict, idxseg, degt_all):
    idx1, seg1, idx2, seg2 = idxseg
    import ml_dtypes

    x = np.asarray(inputs["x"], dtype=np.float32)
    maps = []
    for c in range(cfg.m):
        xb = np.zeros((cfg.bp, CIN), dtype=np.float32)
        xb[: cfg.b] = x[c * cfg.b : (c + 1) * cfg.b]
        xT = np.ascontiguousarray(xb.T).astype(ml_dtypes.bfloat16)
        maps.append(
            {
                "xT_blk": xT,
                "W1": np.asarray(inputs["W1"], dtype=np.float32),
                "b1": np.asarray(inputs["b1"], dtype=np.float32),
                "W2": np.asarray(inputs["W2"], dtype=np.float32),
                "b2": np.asarray(inputs["b2"], dtype=np.float32),
                "deg_t": degt_all[c],
                "idx1": idx1[c],
                "seg1": seg1[c].astype(ml_dtypes.bfloat16),
                "idx2": idx2[c],
                "seg2": seg2[c].astype(ml_dtypes.bfloat16),
            }
        )
    return maps


def kernel(**inputs) -> np.ndarray:
    from concourse import bass_utils

    cfg = Cfg(n=N_NODES, m=N_CORES)
    cfg.finish()
    edge_index = np.asarray(inputs["edge_index"])
    idxseg, degt_all = preprocess(edge_index, cfg)
    nc = build_program(cfg)
    in_maps = make_in_maps(cfg, inputs, idxseg, degt_all)
    res = bass_utils.run_bass_kernel_spmd(nc, in_maps, core_ids=list(range(cfg.m)))
    out = np.concatenate(
        [res.results[c]["y_blk"][: cfg.b] for c in range(cfg.m)], axis=0
    )
    return out.astype(np.float32)



# revision 68
# speedup vs baseline: 1.0063x; 1.0063x over previous
"""ChebNet (K=4, two layers, log_softmax) on 8 Trainium2 NeuronCores.

Self-contained: takes FULL inputs, shards by destination node internally,
runs a single SPMD Bass kernel on cores 0-7, returns the FULL [N, 32]
output.

Math (Horner re-association so propagations happen at output width):
    y = sum_k T_k(L) x @ W[k] + b,  T_k Chebyshev,  L = -D^-1/2 A D^-1/2
      = U0 + L(U1 + L(U2 + L U3)),  U_j = x @ V_j
    V0 = W0 - W2, V1 = W1 - 3 W3, V2 = 2 W2, V3 = 4 W3
Scaled-space recurrence (tables hold S_hat = dis * S, dis = rsqrt degree):
    S_hat3 = dis*U3;  S_hat_j = dis*U_j - dis^2 * (A S_hat_{j+1})
    final: y = U_0 + b - dis * (A S_hat_1)

Per hop, A S_hat is an edge-gather + segment-sum:
    SWDGE dma_gather (8-chunk calls, the HW ring cap, round-robined over
    4 Q7 queues) -> bf16 one-hot (seg == iota) built on VectorE ->
    TensorE bf16 matmul accumulation into PSUM per 128-dest tile.

The gather table for each hop is PIECE-MAJOR across TWO tensors: the
producing hop AllGathers piece 0 (groups 0-6) mid-hop and piece 1 at its
end; gather buckets are piece-aligned, so the consuming hop's bucket-0
calls depend only on piece 0 and are emitted first (bridge), keeping the
Pool engine busy across the hop transition while piece 1's AllGather
finishes.  U_1/U_2 tables live in SBUF as bf16 (U_0, needed only by the
final hop of each layer, round-trips through HBM to free SBUF for a
deeper gather-tile ring); the layer-2 prologue (h^T @ V2) is fused into
layer 1's final writeout.
Gathers are nosync-chained in issue order so the scheduler cannot invert
SWDGE ring-FIFO order or the gather-pool ring release order.
"""

import sys

if "/opt/trn_rl_repo" not in sys.path:
    sys.path.insert(0, "/opt/trn_rl_repo")

import math
from contextlib import ExitStack
from dataclasses import dataclass, field

import numpy as np

P = 128
GCH = 8        # chunks per dma_gather call (65 descs/engine = HW SWDGE ring cap; 9+ hangs)
OB = 16        # one-hot pairs per DVE batch
GT = 8         # dest tiles per psum group
NQ = 4         # SWDGE queues
N_NODES = 100000
N_CORES = 8
CIN, HID, OUT = 128, 64, 32


@dataclass
class LayerSched:
    q: int                 # nodes per 256B gather row
    w: int                 # data width (channels)
    nb: int                # buckets
    buckrows: int          # unused (buckets are piece-aligned; see buckoff)
    buckoff: list = field(default_factory=list)  # per-bucket q-row offsets
    nch: int = 0           # total chunks
    npair: int = 0         # total one-hot pairs
    # per group g: list of calls (bucket, chunk0, glen)
    calls: list = field(default_factory=list)
    # per group g: list of pairs (chunk, tile_local, segcol, par, start, stop)
    pairs: list = field(default_factory=list)


@dataclass
class Cfg:
    n: int
    m: int
    b: int = 0
    t: int = 0
    bp: int = 0
    ng: int = 0
    nch: int = 0
    L: list = field(default_factory=list)  # [LayerSched x2]

    def finish(self):
        assert self.n % self.m == 0
        self.b = self.n // self.m
        self.t = (self.b + P - 1) // P
        self.bp = self.t * P
        self.ng = (self.t + GT - 1) // GT


def pieces(cfg: Cfg):
    """Chunked-AllGather piece boundaries (in dest-row units per core).

    The global gather table is piece-major, ONE TENSOR PER PIECE: piece p's
    AllGather launches as soon as the producing hop finishes groups
    [PG[p], PG[p+1]); gather buckets are piece-aligned, so the consuming
    hop's bucket-p gathers depend only on piece p — they bridge the
    inter-hop bubble."""
    PG = [0, 7, 10, cfg.ng]
    pr = [
        (PG[p] * GT * P, min(PG[p + 1] * GT * P, cfg.bp))
        for p in range(len(PG) - 1)
    ]
    return PG, pr


def _layer_sched(cfg: Cfg, row, col, q, w):
    """Build the edge-stream schedule for one layer (shared across cores)."""
    m, b, bp, t, ng = cfg.m, cfg.b, cfg.bp, cfg.t, cfg.ng

    _, pr = pieces(cfg)
    pstarts = np.array([r0 for r0, _ in pr], dtype=np.int64)
    pends = np.array([r1 for _, r1 in pr], dtype=np.int64)
    pbase = np.concatenate([[0], np.cumsum((pends - pstarts) * m)])

    # bucket 0 = layout piece 0; bucket 1 = layout pieces 1+2 (whose
    # AllGathers are split so the last one is small)
    nb = 2
    buckoff = [0, int(pbase[1]) // q, int(pbase[-1]) // q]
    for p in range(nb):
        assert buckoff[p + 1] - buckoff[p] <= 32767

    ls = LayerSched(q=q, w=w, nb=nb, buckrows=0)
    ls.buckoff = buckoff
    ncell = ng * nb * q

    per_core = []
    counts = np.zeros((m, ncell), dtype=np.int64)
    for c in range(m):
        sel = (row >= c * b) & (row < (c + 1) * b)
        d = (row[sel] - c * b).astype(np.int64)
        s = col[sel].astype(np.int64)
        # piece-major table layout: (piece, core, local row within piece)
        sc = s // b
        rl = s % b
        pidx = np.searchsorted(pends, rl, side="right")
        trow = pbase[pidx] + sc * (pends[pidx] - pstarts[pidx]) + (
            rl - pstarts[pidx]
        )
        prq = trow // q
        par = trow % q
        buck = np.minimum(pidx, 1)
        lidx = prq - np.array(buckoff, dtype=np.int64)[buck]
        tile = d >> 7
        g = tile // GT
        cid = (buck * ng + g) * q + par
        order = np.lexsort((tile, cid))
        per_core.append((d[order], lidx[order], cid[order], tile[order]))
        counts[c] = np.bincount(cid, minlength=ncell)

    kcell = np.array(
        [math.ceil(int(counts[:, i].max()) / P) for i in range(ncell)],
        dtype=np.int64,
    )

    # global chunk layout: cells in cid order; chunk -> (g, buck, par)
    cell_chunk0 = np.concatenate([[0], np.cumsum(kcell)])
    nch = int(cell_chunk0[-1])
    ls.nch = nch

    # per-core slot streams
    S = nch * P
    idx = np.zeros((m, S), dtype=np.int32)
    dloc = np.full((m, S), -1, dtype=np.int64)   # dest local id, -1 pad
    dtile = np.full((m, S), -1, dtype=np.int64)
    for c in range(m):
        d, lidx, cid, tile = per_core[c]
        pos_in_cell = np.arange(d.size) - np.concatenate(
            [[0], np.cumsum(counts[c])]
        )[cid]
        slot = cell_chunk0[cid] * P + pos_in_cell
        idx[c, slot] = lidx
        dloc[c, slot] = d
        dtile[c, slot] = tile

    # calls: bucket-pure GCH windows over each bucket's GLOBALLY contiguous
    # chunk range (cells are bucket-major, so group boundaries don't
    # fragment calls); annotated with the first chunk's group for emission
    # ordering. pairs: per chunk, union tile range over cores, TILE-MAJOR.
    def chunk_group(k):
        cid = int(np.searchsorted(cell_chunk0, k, side="right")) - 1
        return (cid // q) % ng

    ls.flatcalls = []
    for buck in range(nb):
        c0 = int(cell_chunk0[(buck * ng + 0) * q])
        cl_ = (buck * ng + (ng - 1)) * q + q - 1
        c1 = int(cell_chunk0[cl_] + kcell[cl_])
        for w0 in range(c0, c1, GCH):
            glen = min(GCH, c1 - w0)
            ls.flatcalls.append((buck, w0, glen, chunk_group(w0)))

    for g in range(ng):
        gpairs = []
        tcnt = min(GT, t - g * GT)
        for par in range(q):
            for buck in range(nb):
                cid = (buck * ng + g) * q + par
                k0 = int(cell_chunk0[cid])
                k1 = int(cell_chunk0[cid] + kcell[cid])
                for k in range(k0, k1):
                    tl = dtile[:, k * P : (k + 1) * P]
                    real = tl >= 0
                    if not real.any():
                        continue
                    lo = int(tl[real].min())
                    hi = int(tl[real].max())
                    for tt in range(lo, hi + 1):
                        gpairs.append([k, tt - g * GT, par, 0, False, False])
        gpairs.sort(key=lambda e: (e[1], e[0]))
        first = {}
        last = {}
        for i, e in enumerate(gpairs):
            key = e[1]
            if key not in first:
                first[key] = i
            last[key] = i
        for key, i in first.items():
            gpairs[i][4] = True
        for key, i in last.items():
            gpairs[i][5] = True
        ls.pairs.append(gpairs)

    ls.npair = sum(len(p) for p in ls.pairs)
    ls._idx, ls._dloc, ls._dtile = idx, dloc, dtile
    ls._cell_chunk0, ls._kcell = cell_chunk0, kcell

    # per-core seg matrix [P, npair] and idx16 stream
    seg_all, idx_all = [], []
    for c in range(m):
        seg = np.full((P, ls.npair), -1.0, dtype=np.float32)
        colp = 0
        for g in range(ng):
            for k, ttl, par, cc, st, sp in ls.pairs[g]:
                tt = ttl + g * GT
                tl = dtile[c, k * P : (k + 1) * P]
                dl = dloc[c, k * P : (k + 1) * P]
                mask = tl == tt
                seg[mask, colp] = (dl[mask] & 127).astype(np.float32)
                colp += 1
        seg_all.append(np.ascontiguousarray(seg))

        # idx16: slot i -> partition i%16 col i//16, replicated x8
        i16 = idx[c].astype(np.int16).reshape(S // 16, 16).T
        idx_all.append(np.ascontiguousarray(np.tile(i16, (8, 1))))
    return ls, idx_all, seg_all


def preprocess(edge_index: np.ndarray, cfg: Cfg):
    row = np.asarray(edge_index[0], dtype=np.int64)
    col = np.asarray(edge_index[1], dtype=np.int64)
    deg = np.bincount(row, minlength=cfg.n).astype(np.float32)

    l1, idx1, seg1 = _layer_sched(cfg, row, col, q=2, w=HID)
    l2, idx2, seg2 = _layer_sched(cfg, row, col, q=4, w=OUT)
    cfg.L = [l1, l2]
    cfg.nch = l1.nch

    degt_all = []
    for c in range(cfg.m):
        degb = np.zeros(cfg.bp, dtype=np.float32)
        degb[: cfg.b] = deg[c * cfg.b : (c + 1) * cfg.b]
        degt_all.append(np.ascontiguousarray(degb.reshape(cfg.t, P).T))
    return (idx1, seg1, idx2, seg2), degt_all


def build_program(cfg: Cfg):
    import ml_dtypes
    import concourse.bass as bass
    import concourse.tile as tile
    from concourse import bacc, mybir

    f32 = mybir.dt.float32
    bf16 = mybir.dt.bfloat16
    i16 = mybir.dt.int16
    m, b, T, bp, ng = cfg.m, cfg.b, cfg.t, cfg.bp, cfg.ng
    NTAB = m * bp
    l1, l2 = cfg.L

    nc = bacc.Bacc(
        "TRN2", target_bir_lowering=False, debug=False, num_devices=m,
        num_swdge_queues=NQ,
    )

    # ---- I/O ----
    x_t = nc.dram_tensor("xT_blk", [CIN, bp], bf16, kind="ExternalInput")
    w1_t = nc.dram_tensor("W1", [4, CIN, HID], f32, kind="ExternalInput")
    b1_t = nc.dram_tensor("b1", [HID], f32, kind="ExternalInput")
    w2_t = nc.dram_tensor("W2", [4, HID, OUT], f32, kind="ExternalInput")
    b2_t = nc.dram_tensor("b2", [OUT], f32, kind="ExternalInput")
    degt_t = nc.dram_tensor("deg_t", [P, T], f32, kind="ExternalInput")
    idx1_t = nc.dram_tensor("idx1", [P, l1.nch * 8], i16, kind="ExternalInput")
    seg1_t = nc.dram_tensor("seg1", [P, l1.npair], bf16, kind="ExternalInput")
    idx2_t = nc.dram_tensor("idx2", [P, l2.nch * 8], i16, kind="ExternalInput")
    seg2_t = nc.dram_tensor("seg2", [P, l2.npair], bf16, kind="ExternalInput")
    y_t = nc.dram_tensor("y_blk", [bp, OUT], f32, kind="ExternalOutput")

    # ---- internal DRAM: per-piece blk tensors + single tab per hop ----
    # pieces cut the dest-row space at group granularity so each piece's
    # AllGather can launch as soon as its last group's writeout lands,
    # overlapping collective traffic with the producing hop
    PG, _pr = pieces(cfg)
    NPC = len(PG) - 1
    # piece p's table row base (dest-row x m units; piece-major layout)
    pbase = [0]
    for p in range(NPC):
        pbase.append(pbase[-1] + (_pr[p][1] - _pr[p][0]) * m)

    def prows(p):  # dest-row range of piece p
        return _pr[p]

    u0_1 = nc.dram_tensor("U0_1", [bp, HID], bf16)
    u0_2 = nc.dram_tensor("U0_2", [bp, OUT], bf16)
    blks, tabs = {}, {}
    for l, ls in ((1, l1), (2, l2)):
        for j in (3, 2, 1):
            for p in range(NPC):
                r0, r1 = prows(p)
                blks[(l, j, p)] = nc.dram_tensor(
                    f"blk_{l}_{j}_{p}", [(r1 - r0) // ls.q, 128], bf16
                )
            # one tab tensor PER BUCKET: bucket 0 <- piece 0's AllGather;
            # bucket 1 <- pieces 1+2, two AllGathers into adjacent ranges
            tabs[(l, j, 0)] = nc.dram_tensor(
                f"tab_{l}_{j}_0",
                [(prows(0)[1] - prows(0)[0]) * m // ls.q, 128], bf16,
                addr_space="Shared",
            )
            tabs[(l, j, 1)] = nc.dram_tensor(
                f"tab_{l}_{j}_1",
                [(prows(2)[1] - prows(1)[0]) * m // ls.q, 128], bf16,
                addr_space="Shared",
            )

    iota_np = np.broadcast_to(
        np.tile(np.arange(P, dtype=np.float32), OB), (P, OB * P)
    ).astype(ml_dtypes.bfloat16)
    iota_d = nc.inline_tensor(iota_np, name="iota_rep")
    ident_d = nc.inline_tensor(np.eye(P, dtype=np.float32), name="ident")

    with ExitStack() as ctx:
        tc = ctx.enter_context(tile.TileContext(nc, num_cores=m))
        const = ctx.enter_context(tc.tile_pool(name="const", bufs=1))
        xp = ctx.enter_context(tc.tile_pool(name="xp", bufs=2))
        wp = ctx.enter_context(tc.tile_pool(name="wp", bufs=3))
        gp = ctx.enter_context(tc.tile_pool(name="gp", bufs=33))
        up = ctx.enter_context(tc.tile_pool(name="up", bufs=2))
        op = ctx.enter_context(tc.tile_pool(name="op", bufs=3))
        ep = ctx.enter_context(tc.tile_pool(name="ep", bufs=2))
        pst = ctx.enter_context(tc.tile_pool(name="pst", bufs=2, space="PSUM"))
        psu = ctx.enter_context(tc.tile_pool(name="psu", bufs=3, space="PSUM"))
        psa = ctx.enter_context(tc.tile_pool(name="psa", bufs=3, space="PSUM"))

        # ---- constants ----
        iota_s = const.tile([P, OB * P], bf16)
        nc.sync.dma_start(iota_s[:], iota_d[:, :])
        ident_s = const.tile([P, P], f32)
        nc.sync.dma_start(ident_s[:], ident_d[:, :])

        idx_s = const.tile([P, max(l1.nch, l2.nch) * 8], i16)
        nc.sync.dma_start(idx_s[:, : l1.nch * 8], idx1_t[:, :])
        seg1_s = const.tile([P, l1.npair], bf16)
        nc.sync.dma_start(seg1_s[:], seg1_t[:, :])
        seg2_s = const.tile([P, l2.npair], bf16)
        nc.sync.dma_start(seg2_s[:], seg2_t[:, :])

        # U_j (j=1,2) tables resident in SBUF (bf16): [P, 2, T, w]; the
        # U_0 slices live in HBM (u0_l), read back only by the final hop
        U1s = const.tile([P, 2, T, HID], bf16)
        U2s = const.tile([P, 2, T, OUT], bf16)

        # early slice of the layer-2 idx stream (groups 0-1 + g2 bucket 0),
        # loaded at program start into its own tile: the L2 bridge gathers
        # don't wait for the post-layer-1 idx_s reload (WAR on idx_s)
        l2_thresh = int(l2._cell_chunk0[(2 * l2.nb + 0) * l2.q + l2.q - 1]
                        + l2._kcell[(2 * l2.nb + 0) * l2.q + l2.q - 1])
        idx2b = const.tile([P, l2_thresh * 8], i16)
        nc.sync.dma_start(idx2b[:], idx2_t[:, : l2_thresh * 8])

        # V1cat [CIN, 4, HID], V2cat [HID, 4, OUT]
        def vcat(w_t, cl, w):
            ws = const.tile([cl, 4, w], f32)
            nc.sync.dma_start(ws[:], w_t[:, :, :].rearrange("k p c -> p k c"))
            v = const.tile([cl, 4, w], f32)
            nc.vector.tensor_sub(v[:, 0, :], ws[:, 0, :], ws[:, 2, :])
            nc.vector.tensor_scalar(
                out=v[:, 1, :], in0=ws[:, 3, :], scalar1=-3.0, scalar2=None,
                op0=mybir.AluOpType.mult,
            )
            nc.vector.tensor_add(v[:, 1, :], v[:, 1, :], ws[:, 1, :])
            nc.vector.tensor_scalar(
                out=v[:, 2, :], in0=ws[:, 2, :], scalar1=2.0, scalar2=None,
                op0=mybir.AluOpType.mult,
            )
            nc.vector.tensor_scalar(
                out=v[:, 3, :], in0=ws[:, 3, :], scalar1=4.0, scalar2=None,
                op0=mybir.AluOpType.mult,
            )
            return v

        v1f = vcat(w1_t, CIN, HID)
        v2f = vcat(w2_t, HID, OUT)
        v1 = const.tile([CIN, 4, HID], bf16)
        nc.vector.tensor_copy(v1[:], v1f[:])
        v2 = const.tile([HID, 4, OUT], bf16)
        nc.vector.tensor_copy(v2[:], v2f[:])
        ident_b = const.tile([P, P], bf16)
        nc.vector.tensor_copy(ident_b[:], ident_s[:])

        b1s = const.tile([P, HID], f32)
        nc.sync.dma_start(b1s[:1, :], b1_t[:].rearrange("(o c) -> o c", o=1))
        nc.gpsimd.partition_broadcast(b1s[:, :], b1s[:1, :])
        b2s = const.tile([P, OUT], f32)
        nc.sync.dma_start(b2s[:1, :], b2_t[:].rearrange("(o c) -> o c", o=1))
        nc.gpsimd.partition_broadcast(b2s[:, :], b2s[:1, :])

        # ---- dis, -dis, -dis^2 in [P, T]: (p, t) = dest 128t+p ----
        degs = const.tile([P, T], f32)
        nc.sync.dma_start(degs[:], degt_t[:, :])
        dis = const.tile([P, T], f32)
        ndis = const.tile([P, T], f32)
        ndis2 = const.tile([P, T], f32)
        tmp = const.tile([P, T], f32)
        nc.vector.tensor_scalar(
            out=tmp[:], in0=degs[:], scalar1=1.0, scalar2=None,
            op0=mybir.AluOpType.max,
        )
        nc.scalar.activation(tmp[:], tmp[:], mybir.ActivationFunctionType.Sqrt)
        nc.vector.reciprocal(dis[:], tmp[:])
        nc.vector.tensor_scalar(
            out=tmp[:], in0=degs[:], scalar1=0.0, scalar2=None,
            op0=mybir.AluOpType.is_gt,
        )
        nc.vector.tensor_mul(dis[:], dis[:], tmp[:])
        nc.vector.tensor_scalar(
            out=ndis[:], in0=dis[:], scalar1=-1.0, scalar2=None,
            op0=mybir.AluOpType.mult,
        )
        nc.vector.tensor_mul(ndis2[:], dis[:], ndis[:])

        # blk row views: [rows, w] over [rows/q, 128]
        def rows_view(blk, w):
            return blk[:, :].rearrange("r (t c) -> (r t) c", c=w)

        def piece_of_group(g):
            for p in range(NPC):
                if PG[p] <= g < PG[p + 1]:
                    return p
            raise AssertionError(g)

        def allgather_piece(l, j, p, q):
            if p == 0:
                out_ap = tabs[(l, j, 0)].ap().opt()
            else:
                r0a, _ = prows(1)
                r0, r1 = prows(p)
                o0 = (r0 - r0a) * m // q
                o1 = (r1 - r0a) * m // q
                out_ap = tabs[(l, j, 1)][o0:o1, :].opt()
            nc.gpsimd.collective_compute(
                "AllGather",
                mybir.AluOpType.bypass,
                replica_groups=[list(range(m))],
                ins=[blks[(l, j, p)].ap().opt()],
                outs=[out_ap],
            )

        # per-(l, j) piece row views
        BV = {
            (l, j): [
                rows_view(blks[(l, j, p)], w) for p in range(NPC)
            ]
            for (l, w) in ((1, HID), (2, OUT))
            for j in (3, 2, 1)
        }

        def piece_write(l, j, g, tcnt, bt):
            """DMA one group's [P, tcnt, w] rows into its blk piece; kick the
            piece's AllGather when this was the piece's last group."""
            pidx = piece_of_group(g)
            pr0, _ = prows(pidx)
            loc = slice(g * GT * P - pr0, g * GT * P - pr0 + tcnt * P)
            nc.sync.dma_start(
                BV[(l, j)][pidx][loc, :].rearrange("(a p) c -> p a c", p=P),
                bt[:, :tcnt, :],
            )
            if g == PG[pidx + 1] - 1:
                allgather_piece(l, j, pidx, l1.q if l == 1 else l2.q)

        # ---- prologue: U_j = x @ V_j; U1 j=0 (+bias), j=1,2 (dis-scaled)
        #      -> SBUF; j=3 dis-scaled -> blk_1_3 pieces (+AllGather) ----
        def prologue():
            for g in range(ng):
                tcnt = min(GT, T - g * GT)
                xT = xp.tile([CIN, GT * P], bf16, tag="xTd")
                nc.sync.dma_start(
                    xT[:, : tcnt * P],
                    x_t[:, g * GT * P : (g * GT + tcnt) * P],
                )
                btg = ep.tile([P, GT, HID], bf16, tag="bt")
                u0g = up.tile([P, GT, HID], bf16, tag="u0g")
                for a in range(tcnt):
                    k = g * GT + a
                    upsum = psu.tile([P, 4, HID], f32, space="PSUM", tag="upsum")
                    nc.tensor.matmul(
                        out=upsum[:].rearrange("p a c -> p (a c)"),
                        lhsT=xT[:, a * P : (a + 1) * P],
                        rhs=v1[:].rearrange("p a c -> p (a c)"),
                        start=True, stop=True,
                    )
                    nc.vector.tensor_add(
                        u0g[:, a, :], upsum[:, 0, :], b1s[:, :HID]
                    )
                    nc.vector.tensor_scalar(
                        out=U1s[:, 0:2, k, :], in0=upsum[:, 1:3, :],
                        scalar1=dis[:, k : k + 1], scalar2=None,
                        op0=mybir.AluOpType.mult,
                    )
                    nc.vector.tensor_scalar(
                        out=btg[:, a, :], in0=upsum[:, 3, :],
                        scalar1=dis[:, k : k + 1], scalar2=None,
                        op0=mybir.AluOpType.mult,
                    )
                nc.sync.dma_start(
                    u0_1[g * GT * P : (g * GT + tcnt) * P, :].rearrange(
                        "(a p) c -> p a c", p=P
                    ),
                    u0g[:, :tcnt, :],
                )
                piece_write(1, 3, g, tcnt, btg)

        # ---- one hop: consume per-piece tabs (l, jin); produce blk pieces
        #      (l, jout) or the final output ----
        def hop(ls, idx_s, seg_s, jin, Us, jidx, final, l, idx_early=None,
                early_thresh=0):
            w = ls.w
            segbase = 0
            qrr = [0]

            # gather emission order: BRIDGE first — bucket-0 calls (which
            # depend only on piece 0 of the input table, AllGathered mid-way
            # through the PREVIOUS hop) keep the Pool engine busy across the
            # inter-hop transition — then (group, bucket)-major remainder.
            bridge_budget = 24
            order, taken = [], set()
            for ci, (buck, w0, glen, gf) in enumerate(ls.flatcalls):
                if buck == 0 and len(order) < bridge_budget:
                    order.append(ci)
                    taken.add(ci)
            rest = [ci for ci in range(len(ls.flatcalls)) if ci not in taken]
            rest.sort(key=lambda ci: (ls.flatcalls[ci][3], ls.flatcalls[ci][0]))
            order += rest

            gath = {}
            for ci in order:
                buck, w0, glen, gf = ls.flatcalls[ci]
                gt = gp.tile([P, GCH, 128], bf16, tag="gath")
                isrc = (idx_early if idx_early is not None
                        and w0 + glen <= early_thresh else idx_s)
                gi = nc.gpsimd.dma_gather(
                    out_ap=gt[:, :glen, :],
                    in_ap=tabs[(l, jin, buck)][:, :],
                    idxs_ap=isrc[:, w0 * 8 : (w0 + glen) * 8],
                    num_idxs=glen * P,
                    num_idxs_reg=glen * P,
                    elem_size=128,
                    queue_num=qrr[0] % NQ,
                )
                # chain gathers in issue order: keeps engine order consistent
                # with the gp-ring release order (and SWDGE FIFO order)
                if last_gather[0] is not None:
                    gi.ins.add_dependency(
                        last_gather[0],
                        mybir.DependencyInfo(sync=False, no_sync=True),
                    )
                last_gather[0] = gi.ins.name
                qrr[0] += 1
                for j in range(glen):
                    gath[w0 + j] = (gt, j)

            for g in range(ng):
                tcnt = min(GT, T - g * GT)
                psum = psa.tile([P, GT, w], f32, space="PSUM", tag="apsum")
                started = {e[1] for e in ls.pairs[g] if e[4]}
                for ttl in range(tcnt):
                    if ttl not in started:
                        nc.vector.memset(psum[:, ttl, :], 0.0)
                oneh = None
                npair_g = len(ls.pairs[g])
                for i, (k, ttl, par, cc, st, sp) in enumerate(ls.pairs[g]):
                    opos = i % OB
                    if opos == 0:
                        olen = min(OB, npair_g - i)
                        oneh = op.tile([P, OB, P], bf16, tag="oneh")
                        nc.vector.tensor_tensor(
                            out=oneh[:, :olen, :],
                            in0=iota_s[:].rearrange("p (a q) -> p a q", q=P)[
                                :, :olen, :
                            ],
                            in1=seg_s[:, segbase + i : segbase + i + olen]
                            .to_broadcast([P, olen, P]),
                            op=mybir.AluOpType.is_equal,
                        )
                    gt, slot = gath[k]
                    nc.tensor.matmul(
                        out=psum[:, ttl, :],
                        lhsT=oneh[:, opos, :],
                        rhs=gt[:, slot, par * w : (par + 1) * w],
                        start=st, stop=sp,
                    )
                segbase += npair_g

                # ---- writeout ----
                sl = slice(g * GT, g * GT + tcnt)
                wt = wp.tile([P, GT, w], f32, tag="wt")
                nc.vector.tensor_tensor(
                    out=wt[:, :tcnt, :],
                    in0=psum[:, :tcnt, :],
                    in1=(ndis if final else ndis2)[:, sl].to_broadcast(
                        [P, tcnt, w]
                    ),
                    op=mybir.AluOpType.mult,
                )
                if final:
                    u0t = up.tile([P, GT, w], bf16, tag="u0t")
                    nc.sync.dma_start(
                        u0t[:, :tcnt, :],
                        (u0_1 if l == 1 else u0_2)[
                            g * GT * P : (g * GT + tcnt) * P, :
                        ].rearrange("(a p) c -> p a c", p=P),
                    )
                    nc.vector.tensor_add(
                        wt[:, :tcnt, :], wt[:, :tcnt, :], u0t[:, :tcnt, :]
                    )
                else:
                    nc.vector.tensor_add(
                        wt[:, :tcnt, :], wt[:, :tcnt, :],
                        Us[:, jidx - 1, sl, :],
                    )
                if not final:
                    bt = ep.tile([P, GT, w], bf16, tag="bt")
                    nc.vector.tensor_copy(bt[:, :tcnt, :], wt[:, :tcnt, :])
                    piece_write(l, jidx, g, tcnt, bt)
                elif l == 1:
                    nc.vector.tensor_scalar(
                        out=wt[:, :tcnt, :], in0=wt[:, :tcnt, :],
                        scalar1=0.0, scalar2=None, op0=mybir.AluOpType.max,
                    )
                    bt = ep.tile([P, GT, w], bf16, tag="btr")
                    nc.vector.tensor_copy(bt[:, :tcnt, :], wt[:, :tcnt, :])
                    # fused layer-2 prologue: U2_j (SBUF) and blk_2_3 pieces
                    # straight from the relu'd SBUF tiles
                    bt2g = wp.tile([P, GT, OUT], bf16, tag="bt2")
                    u0g2 = up.tile([P, GT, OUT], bf16, tag="u0g2")
                    for a in range(tcnt):
                        k = g * GT + a
                        tp = pst.tile([HID, P], bf16, space="PSUM", tag="tp")
                        nc.tensor.transpose(
                            out=tp[:, :], in_=bt[:, a, :], identity=ident_b[:]
                        )
                        hT = wp.tile([HID, P], bf16, tag="xT")
                        nc.vector.tensor_copy(hT[:], tp[:, :])
                        upsum = psu.tile(
                            [P, 4, OUT], f32, space="PSUM", tag="upsum"
                        )
                        nc.tensor.matmul(
                            out=upsum[:].rearrange("p a c -> p (a c)"),
                            lhsT=hT[:, :],
                            rhs=v2[:].rearrange("p a c -> p (a c)"),
                            start=True, stop=True,
                        )
                        nc.vector.tensor_add(
                            u0g2[:, a, :], upsum[:, 0, :], b2s[:, :OUT]
                        )
                        nc.vector.tensor_scalar(
                            out=U2s[:, 0:2, k, :], in0=upsum[:, 1:3, :],
                            scalar1=dis[:, k : k + 1], scalar2=None,
                            op0=mybir.AluOpType.mult,
                        )
                        nc.vector.tensor_scalar(
                            out=bt2g[:, a, :], in0=upsum[:, 3, :],
                            scalar1=dis[:, k : k + 1], scalar2=None,
                            op0=mybir.AluOpType.mult,
                        )
                    nc.sync.dma_start(
                        u0_2[g * GT * P : (g * GT + tcnt) * P, :].rearrange(
                            "(a p) c -> p a c", p=P
                        ),
                        u0g2[:, :tcnt, :],
                    )
                    piece_write(2, 3, g, tcnt, bt2g)
                else:
                    nc.vector.tensor_copy(lsm[:, sl, :], wt[:, :tcnt, :])

        lsm = const.tile([P, T, OUT], f32)
        last_gather = [None]

        # ================= layer 1 =================
        prologue()
        for j in (2, 1, 0):
            hop(l1, idx_s, seg1_s, j + 1, U1s, j,
                final=(j == 0), l=1)

        # ================= layer 2 =================
        nc.sync.dma_start(idx_s[:, : l2.nch * 8], idx2_t[:, :])
        for j in (2, 1, 0):
            hop(l2, idx_s, seg2_s, j + 1, U2s, j,
                final=(j == 0), l=2, idx_early=idx2b,
                early_thresh=l2_thresh)

        # ---- batched log_softmax epilogue over lsm [P, T, OUT] ----
        red = const.tile([P, T], f32)
        nc.vector.tensor_reduce(
            out=red[:], in_=lsm[:, :, :], axis=mybir.AxisListType.X,
            op=mybir.AluOpType.max,
        )
        nc.vector.tensor_tensor(
            out=lsm[:, :, :], in0=lsm[:, :, :],
            in1=red[:].to_broadcast([P, T, OUT]),
            op=mybir.AluOpType.subtract,
        )
        ex = const.tile([P, T, OUT], bf16)
        nc.scalar.activation(ex[:], lsm[:, :, :], mybir.ActivationFunctionType.Exp)
        nc.vector.tensor_reduce(
            out=red[:], in_=ex[:, :, :], axis=mybir.AxisListType.X,
            op=mybir.AluOpType.add,
        )
        nc.scalar.activation(red[:], red[:], mybir.ActivationFunctionType.Ln)
        nc.vector.tensor_tensor(
            out=lsm[:, :, :], in0=lsm[:, :, :],
            in1=red[:].to_broadcast([P, T, OUT]),
            op=mybir.AluOpType.subtract,
        )
        nc.sync.dma_start(
            y_t[:, :].rearrange("(a p) c -> p a c", p=P), lsm[:, :, :]
        )


    nc.compile()
    return nc


def make_in_maps(cfg: Cfg, inputs: dict, idxseg, degt_all):
    idx1, seg1, idx2, seg2 = idxseg
    import ml_dtypes

    x = np.asarray(inputs["x"], dtype=np.float32)
    maps = []
    for c in range(cfg.m):
        xb = np.zeros((cfg.bp, CIN), dtype=np.float32)
        xb[: cfg.b] = x[c * cfg.b : (c + 1) * cfg.b]
        xT = np.ascontiguousarray(xb.T).astype(ml_dtypes.bfloat16)
        maps.append(
            {
                "xT_blk": xT,
                "W1": np.asarray(inputs["W1"], dtype=np.float32),
                "b1": np.asarray(inputs["b1"], dtype=np.float32),
                "W2": np.asarray(inputs["W2"], dtype=np.float32),
                "b2": np.asarray(inputs["b2"], dtype=np.float32),
                "deg_t": degt_all[c],
                "idx1": idx1[c],
                "seg1": seg1[c].astype(ml_dtypes.bfloat16),
                "idx2": idx2[c],
                "seg2": seg2[c].astype(ml_dtypes.bfloat16),
            }
        )
    return maps


def kernel(**inputs) -> np.ndarray:
    from concourse import bass_utils

    cfg = Cfg(n=N_NODES, m=N_CORES)
    cfg.finish()
    edge_index = np.asarray(inputs["edge_index"])
    idxseg, degt_all = preprocess(edge_index, cfg)
    nc = build_program(cfg)
    in_maps = make_in_maps(cfg, inputs, idxseg, degt_all)
    res = bass_utils.run_bass_kernel_spmd(nc, in_maps, core_ids=list(range(cfg.m)))
    out = np.concatenate(
        [res.results[c]["y_blk"][: cfg.b] for c in range(cfg.m)], axis=0
    )
    return out.astype(np.float32)

